# revision 104
# baseline (speedup 1.0000x reference)
"""FAVOR+ attention (Performer) Trainium2 Bass kernel (v2).

Sharding: token-parallel. 8 cores, core c handles batch c//2, token half c%2
(2048 tokens each). The only cross-core communication is a ~1MB AllReduce of
the per-head kv/denominator statistics over core pairs {0,1},{2,3},{4,5},{6,7}.

Numerics (validated vs reference in fp64/np experiments, tolerance 2e-2):
  - pre-exp path (x, Wqk, Wv, waug, aug tiles) in fp16   -> rel ~2.7e-3
  - post-exp path (phi spill, v, kv, attnT, Wproj) bf16  -> rel ~3.8e-3
  - eps semantics preserved exactly: out = num/(den + 1e-6); den is often
    << 1e-6 so the eps add and the normalized phi_q must be kept.
  - v bias kept on device (folding through eps-division breaks: ~2e-2).

Device-side per core (T=2048 tokens, H=16 heads, D=64, r=256, C=1024):
  pass A (k heads first so the kv AllReduce can start early):
    v_tt   = x_tt @ Wv + bv              token-major [128t, 16h*65] bf16
             (col 64 of each 65-block is a constant 1.0 -> denom row)
    qk^T   = Wqk @ x^T                   [128 dims, T] psum, per m-chunk
    aug_h  = [qk_h + b ; (qk_h + b)^2]   [128, T] f16 (DVE lin + square)
    k head: phi_k = exp(aug^T @ Waug - ln 16)  [128t, 256r] bf16 per tt
            kvT_h[r, 0:65] += phi_k_chunk^T-as-stationary @ [v_h | 1]
            (PE, N=65 bf16, accumulated in psum over tt, DMA-accum over tb)
    q head: phi_q^T = exp(Waug^T @ aug - ln 16) [128r x 2, T] bf16
            -> spilled to DRAM in 4-head groups [128, 4096] bf16
  AllReduce kvT (f32, [128, 2080]) over the batch pair.
  pass B:
    kvaug  = bf16(kvT)                   [128r, 65] slices per (h, rh)
    pn     = kvaug^T @ phi_q^T           [65, T] psum per head (row 64 = den)
    rden   = 1/(den + 1e-6)              (Act Reciprocal w/ float bias; the
             bass helper refuses Reciprocal, emitted directly — validated
             on HW against 1/(x+eps) and end-to-end)
    rb     = partition_broadcast(rden)   [64, T] (Pool)
    attnT  = pn[0:64] * rb               bf16 (DVE)
    out    = attnT^T @ Wproj + bproj     [T, 1024] f32 -> HBM

Known trap encoded below: the gpsimd (SWDGE) accum DMA silently stops
accumulating past 8192 bytes per partition row — kv accum DMAs are split.
"""

import math
import sys

if "/opt/trn_rl_repo" not in sys.path:
    sys.path.insert(0, "/opt/trn_rl_repo")

import numpy as np
import ml_dtypes

import concourse.bacc as bacc
import concourse.mybir as mybir
import concourse.tile as tile

F32 = mybir.dt.float32
F32R = mybir.dt.float32r
F16 = mybir.dt.float16
BF16 = mybir.dt.bfloat16
EXP = mybir.ActivationFunctionType.Exp

ADD = mybir.AluOpType.add
MULT = mybir.AluOpType.mult

H = 16
D = 64
R = 256
C = 1024
QK = 2 * C  # q+k output dims
NCORES = 8
LN_SQRT_R = math.log(math.sqrt(R))  # ln 16
EPS = 1e-6


def _r(ap):
    return ap


def _emit(nc, tc, io, T):
    TBLK = min(512, T)
    NTB = T // TBLK
    TT = TBLK // 128  # 128-token tiles per block

    xT = io["xT"].ap()
    wqkT = io["wqkT"].ap()
    wvT = io["wvT"].ap()
    wprojT = io["wprojT"].ap()
    bqk = io["bqk"].ap()
    bvrow = io["bvrow"].ap()
    bprojrow = io["bprojrow"].ap()
    waug = io["waug"].ap()
    out = io["out"].ap()

    mm = nc.tensor.matmul

    def act_recip(out_ap, in_ap, bias):
        # out = 1/(in + bias) on the Activation engine. bass's helper refuses
        # Reciprocal (accuracy warning); accuracy is validated end-to-end by
        # the rel-err check, so emit the instruction directly.
        eng = nc.scalar
        ins = [
            eng.lower_ap(in_ap),
            mybir.ImmediateValue(dtype=mybir.dt.float32, value=float(bias)),
            mybir.ImmediateValue(dtype=mybir.dt.float32, value=1.0),
            mybir.ImmediateValue(dtype=mybir.dt.float32, value=0.0),
        ]
        return eng.add_instruction(
            mybir.InstActivation(
                name=eng.bass.get_next_instruction_name(),
                func=mybir.ActivationFunctionType.Reciprocal,
                ins=ins,
                outs=[eng.lower_ap(out_ap)],
            )
        )

    with (
        tc.tile_pool(name="consts", bufs=1) as consts,
        tc.tile_pool(name="phq", bufs=3) as phqp,
        tc.tile_pool(name="dram", bufs=1, space="DRAM") as dpool,
    ):
        # ---------------- constants / host-prepped small tensors ----------------
        ebias = consts.tile([128, 1], F32)
        nc.gpsimd.memset(ebias[:], -LN_SQRT_R)
        bqk_sb = consts.tile([128, 16], F32)
        nc.sync.dma_start(bqk_sb[:], bqk[:])
        waug_sb = consts.tile([128, R], F16)
        nc.sync.dma_start(waug_sb[:], waug[:])
        bvr_sb = consts.tile([1, C], F32)
        nc.sync.dma_start(bvr_sb[:], bvrow[:])
        bpr_sb = consts.tile([1, C], F32)
        nc.sync.dma_start(bpr_sb[:], bprojrow[:])

        # broadcast bias rows to [128, C] on the Pool engine (keeps the PE
        # free of DMA-dependent warmup work)
        bvB = consts.tile([128, C], F32)
        bprojB = consts.tile([128, C], F32)
        nc.gpsimd.partition_broadcast(bvB[:], bvr_sb[:])
        nc.gpsimd.partition_broadcast(bprojB[:], bpr_sb[:])

        # DRAM scratch
        phiq_d = dpool.tile([NTB, 128, 16 * 1024], BF16)
        kvin_d = dpool.tile([128, 32 * 65], F32)
        kvout_d = dpool.tile([128, 32 * 65], F32)

        # phi_q load tiles (half-tb = 8 heads each) live at outer scope so
        # pass A can prefetch the first chunks
        def load_phq(tb, half):
            t = phqp.tile([128, 8 * 1024], BF16, tag="phq")
            nc.sync.dma_start(t[:], phiq_d[tb][:, half * 8192 : (half + 1) * 8192])
            return t

        phq_tiles = {}

        # ---------------- pass A ----------------
        with (
            tc.tile_pool(name="wqk", bufs=1) as wqkp,
            tc.tile_pool(name="wv", bufs=1) as wvp,
            tc.tile_pool(name="xsb", bufs=1) as xp,
            tc.tile_pool(name="vt", bufs=2) as vtp,
            tc.tile_pool(name="kvst", bufs=2) as kvstp,
            tc.tile_pool(name="aug", bufs=4) as augp,
            tc.tile_pool(name="phik", bufs=3) as phikp,
            tc.tile_pool(name="sg", bufs=2) as sgp,
            tc.tile_pool(name="ps512", bufs=3, space="PSUM") as qkps,
            tc.tile_pool(name="phi_ps", bufs=2, space="PSUM") as phips,
            tc.tile_pool(name="kv_ps", bufs=1, space="PSUM") as kvps,
        ):
            # weights + x fully resident (fp16); ordered so tb0's k-path
            # dependencies land first
            wv_sb, xsb, wqk_sb = [], [], []
            # first v psum group needs only wv[:, jb0] + x[:, tt0] per chunk;
            # stream loads in dependency order so PE ramps as early as possible
            for c in range(8):
                cs = slice(c * 128, (c + 1) * 128)
                t = wvp.tile([128, C], F16, tag=f"wv{c}", name=f"wv{c}")
                nc.sync.dma_start(t[:, 0:512], wvT[cs, 0:512])
                wv_sb.append(t)
                tx = xp.tile([128, T], F16, tag=f"x{c}", name=f"x{c}")
                nc.scalar.dma_start(tx[:, 0:128], xT[cs, 0:128])
                xsb.append(tx)
            wqk_sb = []
            for c in range(8):
                cs = slice(c * 128, (c + 1) * 128)
                nc.scalar.dma_start(xsb[c][:, 128:TBLK], xT[cs, 128:TBLK])
                nc.sync.dma_start(wv_sb[c][:, 512:C], wvT[cs, 512:C])
            for c in range(8):
                cs = slice(c * 128, (c + 1) * 128)
                tw = wqkp.tile([128, QK], F16, tag=f"wqk{c}", name=f"wqk{c}")
                nc.sync.dma_start(tw[:, C : C + 512], wqkT[cs, C : C + 512])
                wqk_sb.append(tw)
            for c in range(8):
                cs = slice(c * 128, (c + 1) * 128)
                nc.sync.dma_start(wqk_sb[c][:, C + 512 : QK], wqkT[cs, C + 512 : QK])
            for c in range(8):
                cs = slice(c * 128, (c + 1) * 128)
                nc.scalar.dma_start(xsb[c][:, TBLK:T], xT[cs, TBLK:T])
                nc.sync.dma_start(wqk_sb[c][:, 0:C], wqkT[cs, 0:C])

            for tb in range(NTB):
                ts = slice(tb * TBLK, (tb + 1) * TBLK)

                # v tiles: [128t, 16h*65] bf16, col 64 of each 65-block = 1.0
                # (double-buffered per tb to avoid WAR on the last kv matmuls)
                vt = []
                for tt in range(TT):
                    t = vtp.tile([128, H * 65], BF16, tag=f"vt{tt}", name=f"vt{tt}")
                    nc.gpsimd.memset(
                        t[:].rearrange("p (h c) -> p h c", c=65)[:, :, 64:65], 1.0
                    )
                    vt.append(t)

                # ---- v in token-major layout, heads strided by 65
                # (jb-major: jb0 groups only need the first wv half)
                for jb in range(2):
                    for tt in range(TT):
                        t0 = tb * TBLK + tt * 128
                        pv = qkps.tile([128, 512], F32, tag="ps512", name="pv")
                        for c in range(8):
                            mm(
                                pv[:],
                                _r(xsb[c][:, t0 : t0 + 128]),
                                _r(wv_sb[c][:, jb * 512 : (jb + 1) * 512]),
                                start=(c == 0),
                                stop=(c == 7),
                            )
                        dst = vt[tt][:, jb * 8 * 65 : (jb + 1) * 8 * 65].rearrange(
                            "p (h c) -> p h c", c=65
                        )[:, :, 0:64]
                        src = pv[:].rearrange("p (h c) -> p h c", c=64)
                        bias = bvB[:, jb * 512 : (jb + 1) * 512].rearrange(
                            "p (h c) -> p h c", c=64
                        )
                        nc.vector.tensor_tensor(out=dst, in0=src, in1=bias, op=ADD)

                # ---- k heads first (m 8..15), then q heads (m 0..7)
                for m in list(range(8, 16)) + list(range(8)):
                    pqk = qkps.tile([128, TBLK], F32, tag="ps512", name="pqk")
                    for c in range(8):
                        mm(
                            pqk[:],
                            _r(wqk_sb[c][:, m * 128 : (m + 1) * 128]),
                            _r(xsb[c][:, ts]),
                            start=(c == 0),
                            stop=(c == 7),
                        )
                    augE = augp.tile([128, TBLK], F16, tag="augE")
                    augO = augp.tile([128, TBLK], F16, tag="augO")
                    nc.vector.tensor_scalar_add(
                        augE[0:64, :], pqk[0:64, :], bqk_sb[0:64, m : m + 1]
                    )
                    nc.vector.tensor_scalar_add(
                        augO[0:64, :], pqk[64:128, :], bqk_sb[64:128, m : m + 1]
                    )
                    nc.vector.tensor_tensor(
                        out=augE[64:128, :],
                        in0=augE[0:64, :],
                        in1=augE[0:64, :],
                        op=MULT,
                    )
                    nc.vector.tensor_tensor(
                        out=augO[64:128, :],
                        in0=augO[0:64, :],
                        in1=augO[0:64, :],
                        op=MULT,
                    )
                    for idx, aug in ((0, augE), (1, augO)):
                        if m < 8:
                            # q heads: phi_q^T [2*128r, TBLK] -> exp -> spill
                            h = 2 * m + idx
                            g, sl = h // 4, h % 4
                            pphi = phips.tile([128, 2 * TBLK], F32)
                            for rh in range(2):
                                mm(
                                    pphi[:, rh * TBLK : (rh + 1) * TBLK],
                                    _r(waug_sb[:, rh * 128 : (rh + 1) * 128]),
                                    _r(aug[:]),
                                )
                            if sl == 0:
                                sg = sgp.tile([128, 4096], BF16, tag="sg")
                                sg_cur = sg
                            else:
                                sg = sg_cur
                            nc.scalar.activation(
                                sg[:, sl * 1024 : (sl + 1) * 1024],
                                pphi[:],
                                EXP,
                                bias=ebias[:],
                                scale=1.0,
                            )
                            if sl == 3:
                                nc.sync.dma_start(
                                    phiq_d[tb][:, g * 4096 : (g + 1) * 4096], sg[:]
                                )
                                if tb == 0 and g in (1, 3):
                                    # prefetch pass B's first phi_q chunks
                                    half = g // 2
                                    phq_tiles[(0, half)] = load_phq(0, half)
                        else:
                            # k heads: phi_k [128t, 256r] per tt -> kvT accum
                            h = 2 * (m - 8) + idx
                            pphi = phips.tile([128, TT * 256], F32)
                            for tt in range(TT):
                                mm(
                                    pphi[:, tt * 256 : (tt + 1) * 256],
                                    _r(aug[:, tt * 128 : (tt + 1) * 128]),
                                    _r(waug_sb[:]),
                                )
                            phik = phikp.tile([128, TT * 256], BF16, tag="phik")
                            nc.scalar.activation(
                                phik[:], pphi[:], EXP, bias=ebias[:], scale=1.0
                            )
                            # kvT[r, 0:65] per (h, rh): phi_k chunk as stationary
                            if idx == 0:
                                pkv = kvps.tile([128, 260], F32, tag="pkv")
                                pkv_cur = pkv
                            else:
                                pkv = pkv_cur
                            for rh in range(2):
                                od = pkv[:, (idx * 2 + rh) * 65 : (idx * 2 + rh + 1) * 65]
                                for tt in range(TT):
                                    mm(
                                        od,
                                        _r(
                                            phik[
                                                :, tt * 256 + rh * 128 : tt * 256 + (rh + 1) * 128
                                            ]
                                        ),
                                        _r(vt[tt][:, h * 65 : (h + 1) * 65]),
                                        start=(tt == 0),
                                        stop=(tt == TT - 1),
                                    )
                            if idx == 1:
                                if m == 8:
                                    kvst = kvstp.tile(
                                        [128, 32 * 65], F32, tag="kvst", name="kvst"
                                    )
                                    kvst_cur = kvst
                                else:
                                    kvst = kvst_cur
                                nc.scalar.copy(
                                    kvst[:, (m - 8) * 260 : (m - 7) * 260], pkv[:]
                                )
                                if m == 15:
                                    # accumulate kv stats to DRAM. NOTE: the
                                    # swdge accum path silently drops the
                                    # accumulate beyond 8192 bytes per row,
                                    # so split the 8320-byte rows in half.
                                    op = ADD if tb > 0 else mybir.AluOpType.bypass
                                    nc.gpsimd.dma_start(
                                        kvin_d[:, 0:1040],
                                        kvst[:, 0:1040],
                                        accum_op=op,
                                    )
                                    nc.gpsimd.dma_start(
                                        kvin_d[:, 1040:2080],
                                        kvst[:, 1040:2080],
                                        accum_op=op,
                                    )

        # ---------------- kv AllReduce over batch pairs ----------------
        import os as _os

        if _os.environ.get("NO_COLLECTIVE") == "1":
            nc.gpsimd.dma_start(kvout_d[:], kvin_d[:])
        else:
            nc.gpsimd.collective_compute(
                "AllReduce",
                ADD,
                replica_groups=[[0, 1], [2, 3], [4, 5], [6, 7]],
                ins=[kvin_d[:].opt()],
                outs=[kvout_d[:].opt()],
            )

        if "dbg_phiq" in io:
            nc.sync.dma_start(io["dbg_phiq"].ap()[:], phiq_d[:])
            nc.sync.dma_start(io["dbg_kvin"].ap()[:], kvin_d[:])
            nc.sync.dma_start(io["dbg_kvout"].ap()[:], kvout_d[:])

        # ---------------- pass B ----------------
        with (
            tc.tile_pool(name="wproj", bufs=1) as wprojp,
            tc.tile_pool(name="kvsb", bufs=1) as kvsbp,
            tc.tile_pool(name="den", bufs=8) as denp,
            tc.tile_pool(name="rb", bufs=8) as rbp,
            tc.tile_pool(name="attnT", bufs=2) as atp,
            tc.tile_pool(name="outsb", bufs=3) as outp,
            tc.tile_pool(name="num_ps", bufs=5, space="PSUM") as numps,
            tc.tile_pool(name="proj_ps", bufs=3, space="PSUM") as projps,
        ):
            wproj_sb = []
            for c in range(8):
                t = wprojp.tile([128, C], BF16, tag=f"wproj{c}", name=f"wproj{c}")
                nc.sync.dma_start(t[:], wprojT[c * 128 : (c + 1) * 128, :])
                wproj_sb.append(t)

            # single casting DMA (gpsimd can cast): f32 DRAM -> bf16 SBUF
            kvaug = kvsbp.tile([128, 32 * 65], BF16, name="kvaug")
            nc.gpsimd.dma_start(kvaug[:], kvout_d[:])

            def get_phq(tb, half):
                t = phq_tiles.pop((tb, half), None)
                if t is None:
                    t = load_phq(tb, half)
                # keep two chunks of lookahead in flight (bufs=3)
                nh = (tb, half + 1) if half == 0 else (tb + 1, 0)
                nh2 = (nh[0], 1) if nh[1] == 0 else (nh[0] + 1, 0)
                for cand in (nh, nh2):
                    if cand[0] < NTB and cand not in phq_tiles:
                        phq_tiles[cand] = load_phq(*cand)
                return t

            attnT_map = {}

            def emit_nums(tb, hb):
                """num + den chain for heads hb*8 .. hb*8+7 of block tb."""
                if hb == 0:
                    attnT_map[tb] = [
                        atp.tile([128, TBLK], BF16, tag=f"attnT{ct}", name="attnT")
                        for ct in range(8)
                    ]
                attnT = attnT_map[tb]
                phq = get_phq(tb, hb)
                for h in range(hb * 8, hb * 8 + 8):
                    # kvaug slot layout from pass A: m8 = h//2, idx = h%2
                    base = (h // 2) * 260 + (h % 2) * 130
                    hl = h % 8
                    pn = numps.tile([65, TBLK], F32)
                    for rh in range(2):
                        mm(
                            pn[:],
                            _r(kvaug[:, base + rh * 65 : base + (rh + 1) * 65]),
                            _r(
                                phq[:, hl * 1024 + rh * TBLK : hl * 1024 + (rh + 1) * TBLK]
                            ),
                            start=(rh == 0),
                            stop=(rh == 1),
                        )
                    rden = denp.tile([1, TBLK], F32, tag="rden")
                    act_recip(rden[:], pn[64:65, :], EPS)
                    rb = rbp.tile([64, TBLK], F32, tag="rb")
                    nc.gpsimd.partition_broadcast(rb[:], rden[:])
                    ct, half = h // 2, h % 2
                    nc.vector.tensor_tensor(
                        out=attnT[ct][64 * half : 64 * (half + 1), :],
                        in0=pn[0:64, :],
                        in1=rb[:],
                        op=MULT,
                    )

            def emit_proj(tb):
                attnT = attnT_map.pop(tb)
                for tt in range(TT):
                    last = tb == NTB - 1 and tt == TT - 1
                    ot = outp.tile([128, C], F32, tag="outsb")
                    row0 = tb * TBLK + tt * 128
                    for jb in range(2):
                        pp = projps.tile([128, 512], F32)
                        for c in range(8):
                            mm(
                                pp[:],
                                _r(attnT[c][:, tt * 128 : (tt + 1) * 128]),
                                _r(wproj_sb[c][:, jb * 512 : (jb + 1) * 512]),
                                start=(c == 0),
                                stop=(c == 7),
                            )
                        nc.vector.tensor_tensor(
                            out=ot[:, jb * 512 : (jb + 1) * 512],
                            in0=pp[:],
                            in1=bprojB[:, jb * 512 : (jb + 1) * 512],
                            op=ADD,
                        )
                        if last:
                            # split the final tile's store so the drain
                            # overlaps the second half's compute
                            js = slice(jb * 512, (jb + 1) * 512)
                            nc.scalar.dma_start(out[row0 : row0 + 128, js], ot[:, js])
                    if not last:
                        nc.scalar.dma_start(out[row0 : row0 + 128, :], ot[:])

            # software-pipelined: proj(tb) issues after nums(tb+1, half0) so
            # PE fills the den-chain latency with the next block's matmuls
            emit_nums(0, 0)
            emit_nums(0, 1)
            for tb in range(NTB):
                if tb + 1 < NTB:
                    emit_nums(tb + 1, 0)
                emit_proj(tb)
                if tb + 1 < NTB:
                    emit_nums(tb + 1, 1)


def build_program(T, reps=1, timing_mode=False):
    import os as _os

    nc = bacc.Bacc(
        "TRN2", target_bir_lowering=False, debug=False, num_devices=NCORES
    )
    ki = "Internal" if timing_mode else "ExternalInput"
    ko = "Internal" if timing_mode else "ExternalOutput"

    io = {
        "xT": nc.dram_tensor("xT", [C, T], F16, kind=ki),
        "wqkT": nc.dram_tensor("wqkT", [C, QK], F16, kind=ki),
        "wvT": nc.dram_tensor("wvT", [C, C], F16, kind=ki),
        "wprojT": nc.dram_tensor("wprojT", [C, C], BF16, kind=ki),
        "bqk": nc.dram_tensor("bqk", [128, 16], F32, kind=ki),
        "bvrow": nc.dram_tensor("bvrow", [1, C], F32, kind=ki),
        "bprojrow": nc.dram_tensor("bprojrow", [1, C], F32, kind=ki),
        "waug": nc.dram_tensor("waug", [128, R], F16, kind=ki),
        "out": nc.dram_tensor("out", [T, C], F32, kind=ko),
    }
    if _os.environ.get("KERNEL_DEBUG_TAPS") == "1":
        NTB = T // 512
        io["dbg_phiq"] = nc.dram_tensor(
            "dbg_phiq", [NTB, 128, 16 * 1024], BF16, kind="ExternalOutput"
        )
        io["dbg_kvin"] = nc.dram_tensor(
            "dbg_kvin", [128, 32 * 65], F32, kind="ExternalOutput"
        )
        io["dbg_kvout"] = nc.dram_tensor(
            "dbg_kvout", [128, 32 * 65], F32, kind="ExternalOutput"
        )
    if timing_mode:
        dummy = nc.dram_tensor("tdummy", [128, 128], BF16, kind="ExternalOutput")
    with tile.TileContext(nc) as tc:
        if timing_mode:
            # keep-alive output for timing mode; emitted FIRST and reading an
            # input (not `out`) so the probe never serializes after the
            # kernel's final store
            with tc.tile_pool(name="dummyp", bufs=1) as dp:
                dt_ = dp.tile([128, 128], BF16)
                nc.sync.dma_start(dt_[:], io["wprojT"].ap()[0:128, 0:128])
                nc.sync.dma_start(dummy.ap()[:], dt_[:])
        for _ in range(reps):
            _emit(nc, tc, io, T)
    nc.compile()
    return nc


def host_prep(x, Wqkv, bqkv, Wproj, bproj, random_matrix, ncores=NCORES):
    """Build the per-core input maps (all host-side numpy, outside HW timing)."""
    x = np.asarray(x, dtype=np.float32)
    Wqkv = np.asarray(Wqkv, dtype=np.float32)
    bqkv = np.asarray(bqkv, dtype=np.float32)
    Wproj = np.asarray(Wproj, dtype=np.float32)
    bproj = np.asarray(bproj, dtype=np.float32)
    rm = np.asarray(random_matrix, dtype=np.float32)

    B, N, _ = x.shape
    T = B * N // ncores
    halves = N // T if N >= T else 1

    shared = {
        "wqkT": np.ascontiguousarray(Wqkv[:QK].T).astype(np.float16),
        "wvT": np.ascontiguousarray(Wqkv[QK:].T).astype(np.float16),
        "wprojT": np.ascontiguousarray(Wproj.T).astype(ml_dtypes.bfloat16),
        "bqk": np.ascontiguousarray(bqkv[:QK].reshape(16, 128).T),
        "bvrow": np.ascontiguousarray(bqkv[QK:].reshape(1, C)),
        "bprojrow": np.ascontiguousarray(bproj.reshape(1, C)),
        "waug": np.concatenate(
            [rm.T, np.full((64, R), -0.5, np.float32)], axis=0
        ).astype(np.float16),
    }
    in_maps = []
    for core in range(ncores):
        b = core // halves
        half = core % halves
        rows = x[b, half * T : (half + 1) * T, :]
        m = dict(shared)
        m["xT"] = np.ascontiguousarray(rows.T).astype(np.float16)
        in_maps.append(m)
    return in_maps, T


_PROGRAM_CACHE = {}


def kernel(x, Wqkv, bqkv, Wproj, bproj, random_matrix):
    from concourse.bass_utils import run_bass_kernel_spmd

    in_maps, T = host_prep(x, Wqkv, bqkv, Wproj, bproj, random_matrix)
    if T not in _PROGRAM_CACHE:
        _PROGRAM_CACHE[T] = build_program(T)
    nc = _PROGRAM_CACHE[T]
    res = run_bass_kernel_spmd(nc, in_maps, list(range(NCORES)))
    B, N, _ = np.asarray(x).shape
    halves = max(1, N // T)
    out = np.empty((B, N, C), dtype=np.float32)
    for core in range(NCORES):
        b = core // halves
        half = core % halves
        out[b, half * T : (half + 1) * T, :] = res.results[core]["out"]
    return out
